# revision 21
# baseline (speedup 1.0000x reference)
"""Trainium2 Bass kernel for nn_CrossAttention (B=2, N=2048, C=1024, H=16, D=64).

Sharding: 8 cores = 2 batches x 4 head-groups (4 heads each).
Each core computes its head-group's attention + a partial output projection;
the host sums the 4 partials per batch and adds the bias.

v3 (fused): the exp evacuation of softmax scores is the hard serial resource
(ACT, ~1.1us per 128x1024 chunk), so everything else is scheduled into its
shadow. One ACT table set (natural_log_exp) for the whole kernel: rstd is
exp(-0.5*ln(var)), sigmoid gates evacuate as exp(-g), squares run on DVE.

  Phase K: k/v projections (bf16), zero-mean folded into host-centered
      weights, rope in bf16 on DVE, PE transposes into head-paired k^T.
  Phase F: per q block x head pair: row-tiled paired score matmuls, ACT exp
      from 2-bank PSUM (scale=1/8, no max subtraction), col-tiled paired
      attn@v + M=1 ones matmuls for denominators. The q projections for the
      NEXT block, gate projections, and the PREVIOUS block's output
      projection are pumped one small unit at a time between attention
      chunks, all sharing one PSUM bank, so the PE never idles (HAM stays at
      K=8/8) while ACT paces the loop. bf16 partial out, host reduce.
"""

import os
import sys
import numpy as np

for _p in ("/opt/trn_rl_repo", "/opt/pypackages"):
    if _p not in sys.path:
        sys.path.insert(0, _p)

B, N, C = 2, 2048, 1024
H, D = 16, 64
HG = 4            # heads per core
NCH = 16          # token chunks of 128
QB = 4            # q blocks of 512
KTC = 16          # key chunks of 128
EPS = 1e-6

_PROG = None      # cached compiled Bass program
LAST_EXEC_NS = None
LAST_PROFILE = None


def _build_program():
    import concourse.bass as bass
    import concourse.bacc as bacc
    import concourse.tile as tile
    import concourse.mybir as mybir

    F32 = mybir.dt.float32
    BF = mybir.dt.bfloat16
    AF = mybir.ActivationFunctionType
    OP = mybir.AluOpType

    nc = bacc.Bacc("TRN2", target_bir_lowering=False, debug=False, num_devices=8)

    xT = nc.dram_tensor("xT", [8, 128, N], BF, kind="ExternalInput")
    ctxT = nc.dram_tensor("ctxT", [8, 128, N], BF, kind="ExternalInput")
    wq = nc.dram_tensor("wq", [8, 128, 256], BF, kind="ExternalInput")
    wg = nc.dram_tensor("wg", [8, 128, 256], BF, kind="ExternalInput")
    wkv = nc.dram_tensor("wkv", [8, 128, 512], BF, kind="ExternalInput")
    wo = nc.dram_tensor("wo", [2, 128, 1024], BF, kind="ExternalInput")
    cosq = nc.dram_tensor("cosq", [N, D], BF, kind="ExternalInput")
    ssinq = nc.dram_tensor("ssinq", [N, D], BF, kind="ExternalInput")
    cosk = nc.dram_tensor("cosk", [N, D], BF, kind="ExternalInput")
    ssink = nc.dram_tensor("ssink", [N, D], BF, kind="ExternalInput")
    part = nc.dram_tensor("part", [N, C], BF, kind="ExternalOutput")

    def bcast4(ap):
        # [128, 64] -> [128, 4, 64] with step-0 middle dim (read-broadcast)
        return bass.AP(tensor=ap.tensor, offset=ap.offset,
                       ap=[ap.ap[0], [0, 4], ap.ap[1]])

    def bcast_inner(ap, n):
        # [128, 4] -> [128, 4, n] with step-0 inner dim
        return bass.AP(tensor=ap.tensor, offset=ap.offset,
                       ap=[ap.ap[0], ap.ap[1], [0, n]])

    def swap_view(ap):
        # ap: [128, 4, 64] contiguous -> per head read order d+32..d+63, d..d+31
        p, hdim, ddim = ap.ap
        return bass.AP(tensor=ap.tensor, offset=ap.offset + 32 * ddim[0],
                       ap=[p, hdim, [-32 * ddim[0], 2], [ddim[0], 32]])

    with tile.TileContext(nc) as tc:
        import contextlib
        with contextlib.ExitStack() as ctx:
            singles = ctx.enter_context(tc.tile_pool(name="singles", bufs=1))
            slices = ctx.enter_context(tc.tile_pool(name="slices", bufs=2))
            work = ctx.enter_context(tc.tile_pool(name="work", bufs=3))
            persist = ctx.enter_context(tc.tile_pool(name="persist", bufs=1))
            exps_p = ctx.enter_context(tc.tile_pool(name="exps", bufs=6))
            gat_p = ctx.enter_context(tc.tile_pool(name="gat", bufs=2))

            # ---- K-side constants / weights first (needed earliest) ----
            ck_sb = singles.tile([128, NCH, D], BF)
            nc.sync.dma_start(out=ck_sb, in_=cosk.ap().rearrange("(i p) d -> p i d", p=128))
            sk_sb = singles.tile([128, NCH, D], BF)
            nc.sync.dma_start(out=sk_sb, in_=ssink.ap().rearrange("(i p) d -> p i d", p=128))
            wkv_sb = singles.tile([128, 8, 512], BF)
            nc.sync.dma_start(out=wkv_sb, in_=wkv.ap().rearrange("c p f -> p c f"))

            from concourse.masks import make_identity
            identb = singles.tile([128, 128], BF)
            make_identity(nc, identb)
            ones1 = singles.tile([128, 1], BF)
            nc.vector.memset(ones1, 1.0)
            ones2 = singles.tile([128, 64], BF)
            nc.vector.memset(ones2, 1.0)
            eps_sb = singles.tile([128, 1], F32)
            nc.vector.memset(eps_sb, EPS)
            warm = singles.tile([128, 512], BF)
            nc.vector.memset(warm, 0.0)

            # ---- persistent intermediates ----
            pairQ = persist.tile([128, 2, N], BF, tag="pairQ")
            pairK = persist.tile([128, 2, N], BF, tag="pairK")
            v_sb = persist.tile([128, KTC, 4, 64], BF, tag="v_sb")
            sigE = persist.tile([128, 2, N], BF, tag="sigE")   # exp(-gate)
            A_sb = persist.tile([128, 2, N], BF, tag="A_sb")

            pend_tp = []  # transposes deferred to keep PE fed

            def norm_rope(ps, i, cos_t, sin_t, dst_pair):
                """Post-projection chain: variance, rstd=exp(-.5 ln), rope."""
                qview = ps[:, 0:256].rearrange("p (h d) -> p h d", h=4)
                qb = work.tile([128, 4, 64], BF, tag="qb")
                nc.vector.tensor_copy(out=qb, in_=qview)
                sqv = work.tile([128, 4, 64], F32, tag="sq")
                nc.vector.tensor_tensor(out=sqv, in0=qb, in1=qb, op=OP.mult)
                ssum = work.tile([128, 4], F32, tag="ssum")
                nc.vector.tensor_reduce(out=ssum, in_=sqv,
                                        axis=mybir.AxisListType.X, op=OP.add)
                lnv = work.tile([128, 4], F32, tag="lnv")
                nc.scalar.activation(out=lnv, in_=ssum, func=AF.Ln,
                                     scale=1.0 / 64.0, bias=eps_sb)
                rstd = work.tile([128, 4], F32, tag="rstd")
                nc.scalar.activation(out=rstd, in_=lnv, func=AF.Exp,
                                     scale=-0.5)
                qn = work.tile([128, 4, 64], BF, tag="qn")
                nc.vector.tensor_tensor(out=qn, in0=qb,
                                        in1=bcast_inner(rstd, 64), op=OP.mult)
                t1 = work.tile([128, 4, 64], BF, tag="t1")
                nc.vector.tensor_tensor(out=t1, in0=qn, in1=bcast4(cos_t),
                                        op=OP.mult)
                t2 = work.tile([128, 4, 64], BF, tag="t2")
                nc.vector.tensor_tensor(out=t2, in0=swap_view(qn),
                                        in1=bcast4(sin_t), op=OP.mult)
                qr_t = work.tile([128, 4, 64], BF, tag="qr", bufs=3)
                nc.vector.tensor_tensor(out=qr_t, in0=t1, in1=t2, op=OP.add)
                pend_tp.append((qr_t, dst_pair, i))

            def flush_tp(pool, keep=0, copy_act=True, tag="tp"):
                while len(pend_tp) > keep:
                    qr_t, dst_pair, i = pend_tp.pop(0)
                    pst = pool.tile([128, 256], BF, tag=tag, name="pst")
                    for p in range(2):
                        nc.tensor.transpose(
                            pst[:, p * 128:(p + 1) * 128],
                            qr_t[:, 2 * p:2 * p + 2, :].rearrange("p a b -> p (a b)"),
                            identb)
                    dst = dst_pair[:, :, i * 128:(i + 1) * 128]
                    src = pst.rearrange("p (a b) -> p a b", a=2)
                    if copy_act:
                        nc.scalar.copy(out=dst, in_=src)
                    else:
                        nc.vector.tensor_copy(out=dst, in_=src)

            # ================= Phase K: k/v proj / norm / rope ===============
            with tc.tile_pool(name="psA", bufs=3, space="PSUM") as psA, \
                 tc.tile_pool(name="psAt", bufs=2, space="PSUM") as psAt:

                # PE warmup while input DMAs are in flight: keeps HAM busy so
                # the real matmuls start at K=8/8 (2.4 GHz) instead of 1.2.
                for _ in range(14):
                    wps = psA.tile([128, 512], F32, tag="proj", name="wps")
                    nc.tensor.matmul(wps, warm[:, 0:128], warm,
                                     start=True, stop=True)

                for qc in range(4):
                    c_sl = slices.tile([128, 8, 512], BF, tag="slice")
                    nc.sync.dma_start(
                        out=c_sl,
                        in_=ctxT.ap()[:, :, qc * 512:(qc + 1) * 512]
                        .rearrange("c p n -> p c n"))
                    for ns in range(4):
                        j = qc * 4 + ns
                        ps = psA.tile([128, 512], F32, tag="proj")
                        for c in range(8):
                            nc.tensor.matmul(ps,
                                             c_sl[:, c, ns * 128:(ns + 1) * 128],
                                             wkv_sb[:, c, :],
                                             start=(c == 0), stop=(c == 7))
                        flush_tp(psAt, keep=1, copy_act=False)
                        norm_rope(ps, j, ck_sb[:, j, :], sk_sb[:, j, :],
                                  pairK)
                        # v evacuation on ACT
                        nc.scalar.copy(
                            out=v_sb[:, j, :, 0:64],
                            in_=ps[:, 256:512].rearrange("p (h d) -> p h d", h=4))
                flush_tp(psAt, keep=0, copy_act=False)

                # Q-side weights / tables (deferred so K DMAs go first)
                wq_sb = singles.tile([128, 8, 256], BF)
                nc.sync.dma_start(out=wq_sb, in_=wq.ap().rearrange("c p f -> p c f"))
                wg_sb = singles.tile([128, 8, 256], BF)
                nc.sync.dma_start(out=wg_sb, in_=wg.ap().rearrange("c p f -> p c f"))
                cq_sb = singles.tile([128, NCH, D], BF)
                nc.sync.dma_start(out=cq_sb, in_=cosq.ap().rearrange("(i p) d -> p i d", p=128))
                sq_sb = singles.tile([128, NCH, D], BF)
                nc.sync.dma_start(out=sq_sb, in_=ssinq.ap().rearrange("(i p) d -> p i d", p=128))
                wo_sb = singles.tile([128, 2, 1024], BF)
                nc.sync.dma_start(out=wo_sb, in_=wo.ap().rearrange("c p f -> p c f"))

            # ================= Phase F: fused attention ======================
            with tc.tile_pool(name="psSC", bufs=2, space="PSUM") as psSC, \
                 tc.tile_pool(name="psAO", bufs=2, space="PSUM") as psAO, \
                 tc.tile_pool(name="psDN", bufs=1, space="PSUM") as psDN, \
                 tc.tile_pool(name="psX", bufs=1, space="PSUM") as psX:

                def build_q_units(qc):
                    """Q-path of block qc as small PE units pumped between
                    attention chunks (all PSUM via the single psX bank)."""
                    x_sl = slices.tile([128, 8, 512], BF, tag="slice",
                                       name="x_sl")
                    nc.sync.dma_start(
                        out=x_sl,
                        in_=xT.ap()[:, :, qc * 512:(qc + 1) * 512]
                        .rearrange("c p n -> p c n"))
                    units = []
                    qps_box = [None]
                    for ns in range(4):
                        i = qc * 4 + ns

                        def u1(ns=ns):
                            flush_tp(psX, keep=1, copy_act=False, tag="px")
                            qps = psX.tile([128, 256], F32, tag="px",
                                           name="qps")
                            qps_box[0] = qps
                            for c in range(4):
                                nc.tensor.matmul(
                                    qps, x_sl[:, c, ns * 128:(ns + 1) * 128],
                                    wq_sb[:, c, :],
                                    start=(c == 0), stop=False)

                        def u2(ns=ns, i=i):
                            qps = qps_box[0]
                            for c in range(4, 8):
                                nc.tensor.matmul(
                                    qps, x_sl[:, c, ns * 128:(ns + 1) * 128],
                                    wq_sb[:, c, :],
                                    start=False, stop=(c == 7))
                            norm_rope(qps, i, cq_sb[:, i, :], sq_sb[:, i, :],
                                      pairQ)

                        units.append(u1)
                        units.append(u2)
                    for gfc in range(2):
                        def ug(gfc=gfc):
                            flush_tp(psX, keep=1 - gfc, copy_act=False,
                                     tag="px")
                            psg = psX.tile([128, 512], F32, tag="px",
                                           name="psg")
                            for c in range(8):
                                nc.tensor.matmul(
                                    psg, wg_sb[:, c, gfc * 128:(gfc + 1) * 128],
                                    x_sl[:, c, :], start=(c == 0), stop=(c == 7))
                            nc.scalar.activation(
                                out=sigE[:, gfc, qc * 512:(qc + 1) * 512],
                                in_=psg, func=AF.Exp, scale=-1.0)
                        units.append(ug)
                    return units

                def build_po_units(qc):
                    units = []
                    for nk in range(4):
                        for oc in range(2):
                            def up(nk=nk, oc=oc):
                                n1 = qc * 4 + nk
                                po = psX.tile([128, 512], F32, tag="px",
                                              name="po")
                                for fc in range(2):
                                    nc.tensor.matmul(
                                        po,
                                        A_sb[:, fc, n1 * 128:(n1 + 1) * 128],
                                        wo_sb[:, fc, oc * 512:(oc + 1) * 512],
                                        start=(fc == 0), stop=(fc == 1))
                                ev = gat_p.tile([128, 512], BF, tag="ev")
                                nc.vector.tensor_copy(out=ev, in_=po)
                                nc.sync.dma_start(
                                    out=part.ap()[n1 * 128:(n1 + 1) * 128,
                                                  oc * 512:(oc + 1) * 512],
                                    in_=ev)
                            units.append(up)
                    return units

                # Q block 0 must be ready before attention starts
                for u in build_q_units(0):
                    u()

                unitq = []

                def pump():
                    if unitq:
                        unitq.pop(0)()

                for qc in range(4):
                    if qc < 3:
                        unitq.extend(build_q_units(qc + 1))
                    if qc > 0:
                        unitq.extend(build_po_units(qc - 1))
                    qsl = slice(qc * 512, (qc + 1) * 512)
                    for p in range(2):
                        ao_p = psAO.tile([128, 512], F32, tag="ao")
                        dn_p = psDN.tile([128, 512], F32, tag="dn")
                        pend = []  # (expS tile, ktc) awaiting attn MMs

                        def flush_attn(eS, k, ao_p=ao_p, dn_p=dn_p, p=p):
                            # first MM of the k==0 group clears the whole bank;
                            # the second must NOT re-clear (would drop the
                            # first's has_written bits) -> start only on MM1.
                            st = (k == 0)
                            sp = (k == KTC - 1)
                            nc.tensor.matmul(ao_p[0:64, :], v_sb[:, k, 2 * p, :],
                                             eS[:, 0, :], start=st, stop=sp,
                                             tile_position=(0, 0),
                                             skip_group_check=True)
                            nc.tensor.matmul(ao_p[64:128, :], v_sb[:, k, 2 * p + 1, :],
                                             eS[:, 1, :], start=st, stop=sp,
                                             tile_position=(0, 64),
                                             skip_group_check=True)
                            nc.tensor.matmul(dn_p[0:1, :], ones1, eS[:, 0, :],
                                             start=st, stop=sp,
                                             tile_position=(0, 0),
                                             skip_group_check=True)
                            nc.tensor.matmul(dn_p[32:33, :], ones1, eS[:, 1, :],
                                             start=st, stop=sp,
                                             tile_position=(0, 32),
                                             skip_group_check=True)

                        for k in range(KTC):
                            ksl = slice(k * 128, (k + 1) * 128)
                            ps = psSC.tile([128, 1024], F32, tag="sc")
                            nc.tensor.matmul(ps[:, 0:512],
                                             pairK[0:64, p, ksl],
                                             pairQ[0:64, p, qsl],
                                             start=True, stop=True,
                                             tile_position=(0, 0))
                            nc.tensor.matmul(ps[:, 512:1024],
                                             pairK[64:128, p, ksl],
                                             pairQ[64:128, p, qsl],
                                             start=True, stop=True,
                                             tile_position=(64, 0))
                            eS = exps_p.tile([128, 2, 512], BF, tag="expS")
                            nc.scalar.activation(
                                out=eS.rearrange("p a b -> p (a b)"), in_=ps,
                                func=AF.Exp, scale=0.125)
                            pend.append((eS, k))
                            if len(pend) > 2:
                                flush_attn(*pend.pop(0))
                            if k % 2 == 1:
                                pump()
                        for e in pend:
                            flush_attn(*e)

                        # gating: sigmoid(g)/dn with exp(-g) precomputed
                        dns = gat_p.tile([128, 512], BF, tag="dns")
                        nc.vector.tensor_copy(out=dns[0:1, :], in_=dn_p[0:1, :])
                        nc.vector.tensor_copy(out=dns[32:33, :],
                                              in_=dn_p[32:33, :])
                        rbc = psDN.tile([128, 512], F32, tag="dn", name="rbc")
                        nc.tensor.matmul(rbc[0:64, :], ones2[0:1, :], dns[0:1, :],
                                         start=True, stop=True,
                                         tile_position=(0, 0))
                        nc.tensor.matmul(rbc[64:128, :], ones2[32:33, :],
                                         dns[32:33, :], start=True, stop=True,
                                         tile_position=(32, 64))
                        # w = (1 + e^-g) * dn ; A = ao / w
                        w_t = gat_p.tile([128, 512], F32, tag="w")
                        nc.vector.scalar_tensor_tensor(
                            out=w_t, in0=sigE[:, p, qsl], scalar=1.0, op0=OP.add,
                            in1=rbc, op1=OP.mult)
                        rec = gat_p.tile([128, 512], F32, tag="rec")
                        nc.vector.reciprocal_approx_fast(out=rec, in_=w_t)
                        nc.vector.tensor_tensor(out=A_sb[:, p, qsl], in0=ao_p,
                                                in1=rec, op=OP.mult)
                        pump()

                # tail: the last q block's out-proj can't hide under more
                # attention; batch it through the now-free score banks instead
                while unitq:
                    pump()
                for nk in range(4):
                    n1 = 12 + nk
                    po = psSC.tile([128, 1024], F32, tag="sc", name="po")
                    for fc in range(2):
                        for oc in range(2):
                            nc.tensor.matmul(
                                po[:, oc * 512:(oc + 1) * 512],
                                A_sb[:, fc, n1 * 128:(n1 + 1) * 128],
                                wo_sb[:, fc, oc * 512:(oc + 1) * 512],
                                start=(fc == 0), stop=(fc == 1))
                    ev = gat_p.tile([128, 1024], BF, tag="evt")
                    nc.vector.tensor_copy(out=ev, in_=po)
                    nc.sync.dma_start(
                        out=part.ap()[n1 * 128:(n1 + 1) * 128, :], in_=ev)

    nc.compile()
    return nc


def _prep_core(inputs, b, g, bf16):
    x = np.asarray(inputs["x"][b], dtype=np.float32)
    ctx = np.asarray(inputs["context"][b], dtype=np.float32)
    Wq = np.asarray(inputs["Wq"], dtype=np.float32).reshape(H, 2 * D, C)
    Wkv = np.asarray(inputs["Wkv"], dtype=np.float32).reshape(H, 2 * D, C)
    Wo = np.asarray(inputs["Wo"], dtype=np.float32)
    cos = np.asarray(inputs["cos"][b], dtype=np.float32)
    sin = np.asarray(inputs["sin"][b], dtype=np.float32)
    qw = np.asarray(inputs["q_norm_w"], dtype=np.float32)
    kw = np.asarray(inputs["k_norm_w"], dtype=np.float32)

    hs = slice(HG * g, HG * g + HG)
    qr = Wq[hs, :D, :]                       # [4, D, C]
    qr = qr - qr.mean(axis=1, keepdims=True)
    gr = Wq[hs, D:, :]
    kr = Wkv[hs, :D, :]
    kr = kr - kr.mean(axis=1, keepdims=True)
    vr = Wkv[hs, D:, :]

    sgn = np.where(np.arange(D) < D // 2, -1.0, 1.0).astype(np.float32)
    wswap = lambda w: np.concatenate([w[D // 2:], w[:D // 2]])

    return {
        "xT": np.ascontiguousarray(x.T).reshape(8, 128, N).astype(bf16),
        "ctxT": np.ascontiguousarray(ctx.T).reshape(8, 128, N).astype(bf16),
        "wq": np.ascontiguousarray(qr.reshape(HG * D, C).T).reshape(8, 128, 256).astype(bf16),
        "wg": np.ascontiguousarray(gr.reshape(HG * D, C).T).reshape(8, 128, 256).astype(bf16),
        "wkv": np.ascontiguousarray(
            np.concatenate([kr.reshape(HG * D, C), vr.reshape(HG * D, C)], 0).T
        ).reshape(8, 128, 512).astype(bf16),
        "wo": np.ascontiguousarray(
            Wo[:, 256 * g:256 * (g + 1)].T).reshape(2, 128, C).astype(bf16),
        "cosq": (cos * qw[None, :]).astype(bf16),
        "ssinq": (sin * sgn[None, :] * wswap(qw)[None, :]).astype(bf16),
        "cosk": (cos * kw[None, :]).astype(bf16),
        "ssink": (sin * sgn[None, :] * wswap(kw)[None, :]).astype(bf16),
    }


def kernel(**inputs):
    global _PROG, LAST_EXEC_NS, LAST_PROFILE
    import ml_dtypes
    bf16 = ml_dtypes.bfloat16

    if _PROG is None:
        _PROG = _build_program()
    nc = _PROG

    in_maps = [_prep_core(inputs, core // 4, core % 4, bf16) for core in range(8)]

    trace = bool(os.environ.get("BASS_KERNEL_TRACE"))
    kw = {}
    if trace:
        import types
        from trn_agent_boot.trn_boot import _ntff_profile_via_ctypes
        hook = _ntff_profile_via_ctypes('/opt/axon/libaxon_pjrt.so')
        mod = types.ModuleType('antenv.axon_hooks')
        mod.get_axon_ntff_profile_hook = lambda: hook
        sys.modules['antenv.axon_hooks'] = mod
        from concourse import bass_utils
        bass_utils.upload_artifacts = lambda tmpdir: tmpdir
        kw = dict(trace=True, tmpdir=os.environ.get("BASS_KERNEL_TRACE_DIR"))

    from concourse.bass_utils import run_bass_kernel_spmd
    res = run_bass_kernel_spmd(nc, in_maps, core_ids=list(range(8)), **kw)
    LAST_EXEC_NS = res.exec_time_ns
    LAST_PROFILE = res.profile_json

    bo = np.asarray(inputs["bo"], dtype=np.float32)
    out = np.zeros((B, N, C), dtype=np.float32)
    for core in range(8):
        out[core // 4] += np.asarray(res.results[core]["part"], dtype=np.float32)
    out += bo[None, None, :]
    return out
